# revision 33
# baseline (speedup 1.0000x reference)
"""Trainium2 Bass kernel for nn_BindingAffinityPredictor (GNN message passing).

Strategy (8 NeuronCores, SPMD):
- Sort edges by dst; partition nodes into 8 contiguous ranges with ~equal
  edge counts. Each core owns its node range and ALL edges into it, so the
  scatter-add is core-local.
- Math trick: msg_in @ W1 splits into P[src] + Q[dst] + ef@W1c + b1 where
  P = x@W1a, Q = x@W1b are node-level matmuls. The second msg linear
  commutes with segment_sum: agg = segsum(relu(h1)) @ W2 + deg*b2.
- P is exchanged between cores via AllGather. To hide it:
  * P is stored in fp8e4 (scale 16) -- halves collective and gather bytes.
    Final output is a mean over 10k nodes, so fp8 noise averages out
    (measured 5e-4 end-to-end).
  * The AllGather is split in two chunks (slots < / >= W0*64). Each chunk
    is issued as soon as the P rows it covers are written (mid edge-phase
    of the previous layer), and the edge phase is split into two passes:
    pass 0 processes only edges whose src slot is in chunk 0, pass 1 the
    rest. Pass 0 only depends on AG chunk 0, so the second AG chunk
    overlaps pass 0 of the same layer.
- Per window (<=64 dst nodes), each pass has a FIXED 4 edge-tiles of 128
  (padded; SPMD requires identical instruction streams on all cores).
  Per edge tile: one fp8 matmul computes Q[dst]+ef@W1c+b1 (lhsT one-hot
  rows + edge feats + ones), one fp8 identity matmul (entries 1/16) adds
  the gathered 16*P values from PSUM, relu -> fp8 messages, and a one-hot
  scatter matmul accumulates the window's segment sum in PSUM. Pass 0
  evacuates partial sums to SBUF (fp16); pass 1 reloads them via an
  identity matmul and continues the accumulation.
- Node-level matmuls (P/Q/update MLP/readout) run in float32r.
"""

import os
import sys

sys.path.insert(0, "/opt/trn_rl_repo")

import numpy as np

N_NODES = 10000
N_EDGES = 160000
HID = 256
NBOND = 6
NATOM = 62
NLAYERS = 6
N_CORES = 8

WN = 64          # nodes per window
PT0 = 3          # bucket-0 edge tiles per window (src slot < ch0)
PT1 = 5          # bucket-1 edge tiles per window
E0_CAP = PT0 * 128
E1_CAP = PT1 * 128

PSCALE = 16.0    # P stored as fp8(16*P); identity matmul un-scales

_cache: dict = {}


def _f8(x):
    import ml_dtypes
    return np.asarray(x, dtype=ml_dtypes.float8_e4m3)


# ----------------------------------------------------------------------------
# Host-side planning
# ----------------------------------------------------------------------------
def _plan(edge_index: np.ndarray, edge_features: np.ndarray,
          atom_features: np.ndarray):
    src = np.asarray(edge_index[0], dtype=np.int64)
    dst = np.asarray(edge_index[1], dtype=np.int64)
    deg = np.bincount(dst, minlength=N_NODES).astype(np.int64)
    cumdeg = np.concatenate([[0], np.cumsum(deg)])

    bounds = [0]
    for c in range(1, N_CORES):
        bounds.append(int(np.searchsorted(cumdeg, N_EDGES * c / N_CORES)))
    bounds.append(N_NODES)

    # src-bucket threshold: split each core's node range by edge count in
    # proportion to the per-bucket tile budget
    frac = PT0 / (PT0 + PT1)
    thr = np.zeros(N_CORES, dtype=np.int64)
    for c in range(N_CORES):
        mid_edges = (cumdeg[bounds[c]] * (1 - frac)
                     + cumdeg[bounds[c + 1]] * frac)
        thr[c] = int(np.searchsorted(cumdeg, mid_edges))
        thr[c] = min(max(thr[c], bounds[c]), bounds[c + 1])
    core_of_node = np.searchsorted(np.array(bounds[1:]), np.arange(N_NODES),
                                   side="right")
    src_bucket = (src >= thr[core_of_node[src]]).astype(np.int64)

    order = np.argsort(dst, kind="stable")

    # per-node edge counts per bucket
    d0 = np.bincount(dst[src_bucket == 0], minlength=N_NODES)
    d1 = np.bincount(dst[src_bucket == 1], minlength=N_NODES)

    # greedy windows per core with per-bucket caps; forced break at thr[c]
    def pack(c, lo, hi):
        wins = []
        n = lo
        while n < hi:
            start, cnt, e0, e1 = n, 0, 0, 0
            while (n < hi and cnt < WN and e0 + d0[n] <= E0_CAP
                   and e1 + d1[n] <= E1_CAP):
                e0 += d0[n]
                e1 += d1[n]
                cnt += 1
                n += 1
            assert cnt > 0
            wins.append((start, cnt))
        return wins

    halves = []  # [core][half] -> list of (start, cnt)
    for c in range(N_CORES):
        halves.append([pack(c, bounds[c], thr[c]),
                       pack(c, thr[c], bounds[c + 1])])
    W0 = max(len(h[0]) for h in halves)
    W1 = max(len(h[1]) for h in halves)
    W0 += W0 % 2  # ch0 on a 128-row (m-tile) boundary
    nwin = W0 + W1
    nwin += nwin % 2  # S multiple of 128
    W1 = nwin - W0
    S = nwin * WN
    ch0 = W0 * WN                  # slot boundary of AG chunk 0
    T0a = nwin * PT0               # tiles in pass 0
    T0b = nwin * PT1               # tiles in pass 1
    ep = (T0a + T0b) * 128         # padded edge slots per core

    # slot maps
    slot_of = np.full(N_NODES, -1, dtype=np.int64)
    core_slot_of = np.full(N_NODES, -1, dtype=np.int64)
    win_of = np.full(N_NODES, -1, dtype=np.int64)
    for c in range(N_CORES):
        for half, wbase in ((0, 0), (1, W0)):
            for w, (s, cnt) in enumerate(halves[c][half]):
                wi = wbase + w
                slot_of[s:s + cnt] = wi * WN + np.arange(cnt)
                win_of[s:s + cnt] = wi
        core_slot_of[bounds[c]:bounds[c + 1]] = \
            slot_of[bounds[c]:bounds[c + 1]]
    # global gather row in chunk-major p_full:
    #   slot < ch0 : core*ch0 + slot                 (chunk 0, local row)
    #   slot >= ch0: core*(S-ch0) + (slot-ch0)       (chunk 1, local row)
    g_sl = slot_of[src]
    g_co = core_of_node[src]
    grow = np.where(g_sl < ch0,
                    g_co * ch0 + g_sl,
                    g_co * (S - ch0) + (g_sl - ch0))

    per_core = []
    for c in range(N_CORES):
        comb = np.zeros((72, ep), np.float32)
        scat = np.zeros((128, (T0a + T0b) * WN), np.float32)
        srcg = np.zeros(ep, np.int16)
        for b, base_t, pt, cap in ((0, 0, PT0, E0_CAP),
                                   (1, T0a, PT1, E1_CAP)):
            for wi_ in range(nwin):
                # edges of window wi_ in bucket b
                half, w = (0, wi_) if wi_ < W0 else (1, wi_ - W0)
                wl = halves[c][half]
                if w >= len(wl):
                    continue
                s, cnt = wl[w]
                e_ids = order[cumdeg[s]:cumdeg[s + cnt]]
                e_ids = e_ids[src_bucket[e_ids] == b]
                ecnt = len(e_ids)
                assert ecnt <= cap
                if ecnt == 0:
                    continue
                # sort by gather row for locality
                e_ids = e_ids[np.argsort(grow[e_ids], kind="stable")]
                j = np.arange(ecnt)
                pos = (base_t + wi_ * pt) * 128 + j
                dl = dst[e_ids] - s
                comb[0:64][dl, pos] = 1.0
                comb[64:70, pos] = edge_features[e_ids].T
                comb[70, pos] = 1.0
                tt = j // 128
                scat[j % 128, (base_t + wi_ * pt + tt) * WN + dl] = 1.0
                srcg[pos] = grow[e_ids].astype(np.int16)
        wrapped = srcg.reshape(-1, 16).T
        srcw = np.tile(wrapped, (8, 1))   # [128, ep/16]

        af = np.zeros((64, S), np.float32)
        degs = np.zeros((1, S), np.float16)
        mask = np.zeros((1, S), np.float16)
        for half, wbase in ((0, 0), (1, W0)):
            for w, (s, cnt) in enumerate(halves[c][half]):
                sl = slice((wbase + w) * WN, (wbase + w) * WN + cnt)
                af[:NATOM, sl] = atom_features[s:s + cnt].T
                degs[0, sl] = deg[s:s + cnt]
                mask[0, sl] = 1.0
        per_core.append(dict(comb=_f8(comb), scat=_f8(scat), srcw=srcw,
                             af=af, deg=degs, mask=mask))

    return dict(nwin=nwin, W0=W0, slots=S, ch0=ch0, T0a=T0a, T0b=T0b,
                ep=ep, per_core=per_core, bounds=bounds, halves=halves)


def _pack_weights(inp):
    f32 = np.float32
    L = NLAYERS
    w1 = np.asarray(inp["msg_w1"], f32)   # [L, 518, 256]
    b1 = np.asarray(inp["msg_b1"], f32)
    w2 = np.asarray(inp["msg_w2"], f32)
    b2 = np.asarray(inp["msg_b2"], f32)
    u1 = np.asarray(inp["upd_w1"], f32)
    ub1 = np.asarray(inp["upd_b1"], f32)
    u2 = np.asarray(inp["upd_w2"], f32)
    ub2 = np.asarray(inp["upd_b2"], f32)

    # W1AB [L, 128, 1024]: col k2*512 + 0:256 = w1a, +256:512 = w1b
    w1ab = np.zeros((L, 128, 1024), f32)
    for k2 in range(2):
        w1ab[:, :, k2 * 512:k2 * 512 + 256] = w1[:, k2 * 128:(k2 + 1) * 128, :]
        w1ab[:, :, k2 * 512 + 256:(k2 + 1) * 512] = \
            w1[:, 256 + k2 * 128:256 + (k2 + 1) * 128, :]

    # W1CB [L, 8, 256] fp8: rows 0:6 = w1c, row 6 = b1, row 7 = 0
    w1cb = np.zeros((L, 8, 256), np.float32)
    w1cb[:, 0:6] = w1[:, 512:518, :]
    w1cb[:, 6] = b1

    # WU [L, 128, 2048]
    wu = np.zeros((L, 128, 2048), f32)

    def put(off, m):
        for k2 in range(2):
            for j2 in range(2):
                blk = off + (k2 * 2 + j2) * 128
                wu[:, :, blk:blk + 128] = \
                    m[:, k2 * 128:(k2 + 1) * 128, j2 * 128:(j2 + 1) * 128]
    put(0, w2)
    put(512, u1[:, 0:256, :])
    put(1024, u1[:, 256:512, :])
    put(1536, u2)

    bias = np.zeros((L, 1, 1024), np.float16)
    bias[:, 0, 0:256] = b2
    bias[:, 0, 256:512] = ub1
    bias[:, 0, 512:768] = ub2

    embw = np.zeros((64, 256), f32)
    embw[:NATOM] = np.asarray(inp["embed_w"], f32)
    embb = np.asarray(inp["embed_b"], np.float16).reshape(1, 256)

    rw1 = np.asarray(inp["r_w1"], f32)
    rw1p = np.zeros((128, 512), f32)
    for k2 in range(2):
        for j2 in range(2):
            rw1p[:, (k2 * 2 + j2) * 128:(k2 * 2 + j2 + 1) * 128] = \
                rw1[k2 * 128:(k2 + 1) * 128, j2 * 128:(j2 + 1) * 128]
    rb1 = np.asarray(inp["r_b1"], np.float16).reshape(1, 256)
    rw2 = np.asarray(inp["r_w2"], f32)
    rw2p = np.zeros((128, 256), f32)
    for k2 in range(2):
        rw2p[:, k2 * 128:(k2 + 1) * 128] = rw2[k2 * 128:(k2 + 1) * 128, :]
    rb2 = np.asarray(inp["r_b2"], np.float16).reshape(1, 128)
    rw3 = np.asarray(inp["r_w3"], np.float16).reshape(128, 1)

    return dict(w1ab=w1ab, w1cb=_f8(w1cb), wu=wu, bias=bias, embw=embw,
                embb=embb, rw1=rw1p, rb1=rb1, rw2=rw2p, rb2=rb2, rw3=rw3)


# ----------------------------------------------------------------------------
# Device program
# ----------------------------------------------------------------------------
def _build(nwin: int, W0: int, nlayers: int = NLAYERS,
           skip_edge: bool = False, skip_gather: bool = False,
           skip_coll: bool = False, reps: int = 1):
    import concourse.bacc as bacc
    import concourse.mybir as mybir
    import concourse.tile as tile
    from concourse.masks import make_identity

    dt = mybir.dt
    AF = mybir.ActivationFunctionType
    ALU = mybir.AluOpType

    S = nwin * WN
    ch0 = W0 * WN
    assert ch0 % 128 == 0
    T0a = nwin * PT0
    T0b = nwin * PT1
    ep = (T0a + T0b) * 128

    def _split(c0, n):
        """Cover [c0, c0+n) with pieces of width 256..512 (f32r matmuls
        need N >= 256 for full PE speed; n is a multiple of 128)."""
        out = []
        while n > 0:
            if n >= 768 or n == 512:
                w = 512
            elif n == 640:
                w = 384
            elif n in (256, 384):
                w = n
            else:
                raise AssertionError(f"bad segment {n}")
            out.append((c0, w))
            c0 += w
            n -= w
        return out

    # node chunks aligned to the AG boundary so AG chunk 0 can issue as
    # soon as the node chunks covering [0, ch0) are done
    chunks = _split(0, ch0) + _split(ch0, S - ch0)
    NCH = len(chunks)
    MT = S // 128

    nc = bacc.Bacc("TRN2", target_bir_lowering=False, debug=False,
                   num_swdge_queues=4)

    # ---- I/O ----
    comb_in = nc.dram_tensor("comb", [72, ep], dt.float8e4,
                             kind="ExternalInput")
    scat_in = nc.dram_tensor("scat", [128, (T0a + T0b) * WN], dt.float8e4,
                             kind="ExternalInput")
    srcw_in = nc.dram_tensor("srcw", [128, ep // 16], dt.int16,
                             kind="ExternalInput")
    af_in = nc.dram_tensor("af", [64, S], dt.float32r, kind="ExternalInput")
    deg_in = nc.dram_tensor("deg", [1, S], dt.float16, kind="ExternalInput")
    mask_in = nc.dram_tensor("mask", [1, S], dt.float16, kind="ExternalInput")
    w1ab_in = nc.dram_tensor("w1ab", [NLAYERS, 128, 1024], dt.float32r,
                             kind="ExternalInput")
    w1cb_in = nc.dram_tensor("w1cb", [NLAYERS, 8, 256], dt.float8e4,
                             kind="ExternalInput")
    wu_in = nc.dram_tensor("wu", [NLAYERS, 128, 2048], dt.float32r,
                           kind="ExternalInput")
    bias_in = nc.dram_tensor("bias", [NLAYERS, 1, 1024], dt.float16,
                             kind="ExternalInput")
    embw_in = nc.dram_tensor("embw", [64, 256], dt.float32r,
                             kind="ExternalInput")
    embb_in = nc.dram_tensor("embb", [1, 256], dt.float16,
                             kind="ExternalInput")
    rw1_in = nc.dram_tensor("rw1", [128, 512], dt.float32r,
                            kind="ExternalInput")
    rb1_in = nc.dram_tensor("rb1", [1, 256], dt.float16, kind="ExternalInput")
    rw2_in = nc.dram_tensor("rw2", [128, 256], dt.float32r,
                            kind="ExternalInput")
    rb2_in = nc.dram_tensor("rb2", [1, 128], dt.float16, kind="ExternalInput")
    rw3_in = nc.dram_tensor("rw3", [128, 1], dt.float16, kind="ExternalInput")

    partial_out = nc.dram_tensor("partial", [1, 1], dt.float32,
                                 kind="ExternalOutput")

    # internal DRAM: P table, chunk-major fp8
    p_mine = nc.dram_tensor("p_mine", [S, HID], dt.float8e4)
    p_full = nc.dram_tensor("p_full", [N_CORES * S, HID], dt.float8e4,
                            addr_space="Shared")

    with tile.TileContext(nc) as tc:
        with (
            tc.tile_pool(name="const", bufs=1) as cpool,
            tc.tile_pool(name="state", bufs=1) as spool,
            tc.tile_pool(name="wstage", bufs=2) as wpool,
            tc.tile_pool(name="gather",
                         bufs=(max(T0a, T0b) + 15) // 16 + 1) as gpool,
            tc.tile_pool(name="ework", bufs=3) as epool,
            tc.tile_pool(name="nwork", bufs=2) as npool,
            tc.tile_pool(name="epsum", bufs=3, space="PSUM") as epsum,
            tc.tile_pool(name="rpsum", bufs=2, space="PSUM") as rpsum,
            tc.tile_pool(name="npsum", bufs=3, space="PSUM") as npsum,
        ):
            # ---- resident constants ----
            comb_sb = cpool.tile([72, ep], dt.float8e4, tag="comb")
            nc.sync.dma_start(out=comb_sb[:], in_=comb_in[:])
            scat_sb = cpool.tile([128, (T0a + T0b) * WN], dt.float8e4,
                                 tag="scat")
            nc.sync.dma_start(out=scat_sb[:], in_=scat_in[:])
            srcw_sb = cpool.tile([128, ep // 16], dt.int16, tag="srcw")
            nc.sync.dma_start(out=srcw_sb[:], in_=srcw_in[:])
            af_sb = cpool.tile([64, S], dt.float32r, tag="af")
            nc.sync.dma_start(out=af_sb[:], in_=af_in[:])
            deg_sb = cpool.tile([1, S], dt.float16, tag="deg")
            nc.sync.dma_start(out=deg_sb[:], in_=deg_in[:])
            mask_sb = cpool.tile([1, S], dt.float16, tag="mask")
            nc.sync.dma_start(out=mask_sb[:], in_=mask_in[:])
            embw_sb = cpool.tile([64, 256], dt.float32r, tag="embw")
            nc.sync.dma_start(out=embw_sb[:], in_=embw_in[:])
            embb_sb = cpool.tile([1, 256], dt.float16, tag="embb")
            nc.sync.dma_start(out=embb_sb[:], in_=embb_in[:])
            rw1_sb = cpool.tile([128, 512], dt.float32r, tag="rw1")
            nc.sync.dma_start(out=rw1_sb[:], in_=rw1_in[:])
            rb1_sb = cpool.tile([1, 256], dt.float16, tag="rb1")
            nc.sync.dma_start(out=rb1_sb[:], in_=rb1_in[:])
            rw2_sb = cpool.tile([128, 256], dt.float32r, tag="rw2")
            nc.sync.dma_start(out=rw2_sb[:], in_=rw2_in[:])
            rb2_sb = cpool.tile([1, 128], dt.float16, tag="rb2")
            nc.sync.dma_start(out=rb2_sb[:], in_=rb2_in[:])
            rw3_sb = cpool.tile([128, 1], dt.float16, tag="rw3")
            nc.sync.dma_start(out=rw3_sb[:], in_=rw3_in[:])
            ident = cpool.tile([128, 128], dt.float32, tag="ident")
            make_identity(nc, ident[:])
            ident16 = cpool.tile([128, 128], dt.float16, tag="ident16")
            nc.vector.tensor_copy(out=ident16[:], in_=ident[:])
            # fp8 identity scaled by 1/PSCALE: un-scales the gathered 16*P
            ident8 = cpool.tile([128, 128], dt.float8e4, tag="ident8")
            nc.scalar.activation(ident8[:], ident[:], AF.Copy,
                                 scale=1.0 / PSCALE)

            # ---- persistent state (feature-major, col = k2*S + slot) ----
            x_a = spool.tile([128, 2 * S], dt.float32r, tag="x_a")
            x_b = spool.tile([128, 2 * S], dt.float32r, tag="x_b")
            rh_fm = spool.tile([128, 2 * S], dt.float32r, tag="rh_fm")
            agg_fm = spool.tile([128, 2 * S], dt.float32r, tag="agg_fm")
            h_fm = spool.tile([128, 2 * S], dt.float32r, tag="h_fm")
            wr_a = spool.tile([72, nwin * 256], dt.float8e4, tag="wr_a")
            agg0 = spool.tile([64, nwin * 256], dt.float16, tag="agg0")
            x_ab = [x_a, x_b]
            wr_ab = [wr_a, wr_a]

            def fm(t, k2, c0_, n):
                return t[:, k2 * S + c0_:k2 * S + c0_ + n]

            # node chunk ci completes once pass-1 window
            # (cc+cw)//WN - 1 (+LAG) is evicted
            LAG = int(os.environ.get("KERNEL_LAG", "0"))
            wend_chunks = {}
            for ci, (cc, cw) in enumerate(chunks):
                wend = min((cc + cw) // WN - 1 + LAG, nwin - 1)
                wend_chunks.setdefault(wend, []).append(ci)
            # AG chunk 0 is emitted after the node chunk covering slot ch0-1
            ag0_chunk = next(ci for ci, (cc, cw) in enumerate(chunks)
                             if cc <= ch0 - 1 < cc + cw)

            def stage_weights(layer):
                w1ab_sb = wpool.tile([128, 1024], dt.float32r, tag="w1ab")
                nc.sync.dma_start(out=w1ab_sb[:], in_=w1ab_in[layer])
                wu_sb = wpool.tile([128, 2048], dt.float32r, tag="wu")
                nc.sync.dma_start(out=wu_sb[:], in_=wu_in[layer])
                bias_sb = wpool.tile([1, 1024], dt.float16, tag="bias")
                nc.sync.dma_start(out=bias_sb[:], in_=bias_in[layer])
                return w1ab_sb, wu_sb, bias_sb

            def pq_chunk(layer, cc, cw, x_src, w1ab_sb, wrhs):
                for m in range(cc // 128, (cc + cw) // 128):
                    ps = npsum.tile([128, 512], dt.float32, tag="nps")
                    for k2 in range(2):
                        nc.tensor.matmul(
                            out=ps[:, 0:256],
                            lhsT=x_src[:, k2 * S + m * 128:
                                       k2 * S + (m + 1) * 128],
                            rhs=w1ab_sb[:, k2 * 512:k2 * 512 + 256],
                            start=(k2 == 0), stop=(k2 == 1))
                    p8 = npool.tile([128, 256], dt.float8e4, tag="p8")
                    nc.scalar.activation(p8[:], ps[:, 0:256], AF.Copy,
                                         scale=PSCALE)
                    nc.sync.dma_start(
                        out=p_mine[m * 128:(m + 1) * 128, :], in_=p8[:])
                for w in range(cc // WN, (cc + cw) // WN):
                    qs = rpsum.tile([64, 256], dt.float32, tag="rps")
                    for k2 in range(2):
                        nc.tensor.matmul(
                            out=qs[:],
                            lhsT=x_src[:, k2 * S + w * WN:
                                       k2 * S + (w + 1) * WN],
                            rhs=w1ab_sb[:, k2 * 512 + 256:(k2 + 1) * 512],
                            start=(k2 == 0), stop=(k2 == 1))
                    nc.vector.tensor_copy(
                        out=wrhs[0:64, w * 256:(w + 1) * 256], in_=qs[:])
                    nc.sync.dma_start(
                        out=wrhs[64:72, w * 256:(w + 1) * 256],
                        in_=w1cb_in[layer])

            def node_chunk(layer, cc, cw, x_cur, x_nxt, wu_sb, bias_sb):
                for j2 in range(2):
                    ps = npsum.tile([128, 512], dt.float32, tag="nps")
                    for k2 in range(2):
                        blk = (k2 * 2 + j2) * 128
                        nc.tensor.matmul(
                            out=ps[:, :cw], lhsT=wu_sb[:, blk:blk + 128],
                            rhs=fm(rh_fm, k2, cc, cw),
                            start=(k2 == 0), stop=False)
                    nc.tensor.matmul(
                        out=ps[:, :cw],
                        lhsT=bias_sb[:, j2 * 128:(j2 + 1) * 128],
                        rhs=deg_sb[:, cc:cc + cw],
                        start=False, stop=True)
                    nc.scalar.copy(out=fm(agg_fm, j2, cc, cw), in_=ps[:, :cw])
                for j2 in range(2):
                    ps = npsum.tile([128, 512], dt.float32, tag="nps")
                    for k2 in range(2):
                        blk = 512 + (k2 * 2 + j2) * 128
                        nc.tensor.matmul(
                            out=ps[:, :cw], lhsT=wu_sb[:, blk:blk + 128],
                            rhs=fm(x_cur, k2, cc, cw),
                            start=(k2 == 0), stop=False)
                    for k2 in range(2):
                        blk = 1024 + (k2 * 2 + j2) * 128
                        nc.tensor.matmul(
                            out=ps[:, :cw], lhsT=wu_sb[:, blk:blk + 128],
                            rhs=fm(agg_fm, k2, cc, cw),
                            start=False, stop=False)
                    nc.tensor.matmul(
                        out=ps[:, :cw],
                        lhsT=bias_sb[:, 256 + j2 * 128:256 + (j2 + 1) * 128],
                        rhs=mask_sb[:, cc:cc + cw],
                        start=False, stop=True)
                    nc.scalar.activation(
                        fm(h_fm, j2, cc, cw), ps[:, :cw], AF.Relu)
                for j2 in range(2):
                    ps = npsum.tile([128, 512], dt.float32, tag="nps")
                    for k2 in range(2):
                        blk = 1536 + (k2 * 2 + j2) * 128
                        nc.tensor.matmul(
                            out=ps[:, :cw], lhsT=wu_sb[:, blk:blk + 128],
                            rhs=fm(h_fm, k2, cc, cw),
                            start=(k2 == 0), stop=False)
                    nc.tensor.matmul(
                        out=ps[:, :cw],
                        lhsT=bias_sb[:, 512 + j2 * 128:512 + (j2 + 1) * 128],
                        rhs=mask_sb[:, cc:cc + cw],
                        start=False, stop=True)
                    nc.scalar.copy(out=fm(x_nxt, j2, cc, cw), in_=ps[:, :cw])

            def readout_chunk(ci, cc, cw, x_fin, vred):
                for j2 in range(2):
                    ps = npsum.tile([128, 512], dt.float32, tag="nps")
                    for k2 in range(2):
                        blk = (k2 * 2 + j2) * 128
                        nc.tensor.matmul(
                            out=ps[:, :cw], lhsT=rw1_sb[:, blk:blk + 128],
                            rhs=fm(x_fin, k2, cc, cw),
                            start=(k2 == 0), stop=False)
                    nc.tensor.matmul(
                        out=ps[:, :cw],
                        lhsT=rb1_sb[:, j2 * 128:(j2 + 1) * 128],
                        rhs=mask_sb[:, cc:cc + cw],
                        start=False, stop=True)
                    nc.scalar.activation(
                        fm(rh_fm, j2, cc, cw), ps[:, :cw], AF.Relu)
                ps = npsum.tile([128, 512], dt.float32, tag="nps")
                for k2 in range(2):
                    nc.tensor.matmul(
                        out=ps[:, :cw],
                        lhsT=rw2_sb[:, k2 * 128:(k2 + 1) * 128],
                        rhs=fm(rh_fm, k2, cc, cw),
                        start=(k2 == 0), stop=False)
                nc.tensor.matmul(
                    out=ps[:, :cw], lhsT=rb2_sb[:],
                    rhs=mask_sb[:, cc:cc + cw],
                    start=False, stop=True)
                h2 = npool.tile([128, 512], dt.float16, tag="h2")
                nc.scalar.activation(h2[:, :cw], ps[:, :cw], AF.Relu)
                vp = npsum.tile([128, 512], dt.float32, tag="nps")
                nc.tensor.matmul(
                    out=vp[0:1, :cw], lhsT=rw3_sb[:], rhs=h2[:, :cw],
                    start=True, stop=True)
                nc.vector.tensor_reduce(
                    out=vred[:, ci:ci + 1], in_=vp[0:1, :cw],
                    axis=mybir.AxisListType.X, op=ALU.add)

            def allgather(cix):
                if skip_coll:
                    return
                if cix == 0:
                    nc.gpsimd.collective_compute(
                        "AllGather", ALU.bypass,
                        replica_groups=[list(range(N_CORES))],
                        ins=[p_mine[0:ch0]],
                        outs=[p_full[0:N_CORES * ch0]])
                else:
                    nc.gpsimd.collective_compute(
                        "AllGather", ALU.bypass,
                        replica_groups=[list(range(N_CORES))],
                        ins=[p_mine[ch0:S]],
                        outs=[p_full[N_CORES * ch0:N_CORES * S]])

            # queue_num must track the global SWDGE-DMA emission order mod 4:
            # tile assigns DMASW sem lanes round-robin per SWDGE DMA, and the
            # sim/hw locks each lane to one queue.
            qctr = [0]

            def edge_pass(layer, b, wrhs, interleave):
                """Pass b: bucket-b tiles of every window.

                interleave: callable(window_idx) emitting node work after a
                window completes (pass 1 only)."""
                if b == 0:
                    base_t, pt, tp_ = 0, PT0, T0a
                    region = p_full[0:N_CORES * ch0]
                else:
                    # AG chunk 1 is emitted here -- after pass 0's gather
                    # descgens on the Pool queue, so they are not stuck
                    # behind its engine hold -- and before its readers.
                    allgather(1)
                    base_t, pt, tp_ = T0a, PT1, T0b
                    region = p_full[N_CORES * ch0:N_CORES * S]
                ngroups = (tp_ + 15) // 16
                pg_tiles = []
                for g in range(ngroups):
                    tlo = g * 16
                    tcnt = min(16, tp_ - tlo)
                    pg = gpool.tile([128, 16, 256], dt.float8e4, tag="pg")
                    if skip_gather:
                        nc.gpsimd.memset(pg[:], 0)
                    else:
                        col0 = (base_t + tlo) * 8
                        qn = (tcnt + 3) // 4    # 512-idx quarters
                        for hf in range(qn):
                            idx_n = min(512, tcnt * 128 - hf * 512)
                            nc.gpsimd.dma_gather(
                                pg[:, hf * 4:hf * 4 + idx_n // 128, :],
                                region,
                                srcw_sb[:, col0 + hf * 32:
                                        col0 + hf * 32 + idx_n // 16],
                                idx_n, idx_n, 256, single_packet=False,
                                queue_num=qctr[0] % 4)
                            qctr[0] += 1
                    pg_tiles.append(pg)
                for w in range(nwin):
                    rps = rpsum.tile([64, 256], dt.float32, tag="rps")
                    if b == 1:
                        nc.tensor.matmul(
                            out=rps[:], lhsT=ident16[0:64, 0:64],
                            rhs=agg0[:, w * 256:(w + 1) * 256],
                            start=True, stop=False)
                    for t in range(pt):
                        tl = w * pt + t          # pass-local tile
                        g = base_t + tl          # global tile
                        hps = epsum.tile([128, 256], dt.float32, tag="hps")
                        nc.tensor.matmul(
                            out=hps[:],
                            lhsT=comb_sb[:, g * 128:(g + 1) * 128],
                            rhs=wrhs[:, w * 256:(w + 1) * 256],
                            start=True, stop=False)
                        nc.tensor.matmul(
                            out=hps[:], lhsT=ident8[:],
                            rhs=pg_tiles[tl // 16][:, tl % 16, :],
                            start=False, stop=True)
                        r8 = epool.tile([128, 256], dt.float8e4, tag="r8")
                        if g % 2 == 0:
                            nc.scalar.activation(r8[:], hps[:], AF.Relu)
                        else:
                            nc.vector.tensor_scalar_max(r8[:], hps[:], 0.0)
                        nc.tensor.matmul(
                            out=rps[:],
                            lhsT=scat_sb[:, g * WN:(g + 1) * WN],
                            rhs=r8[:],
                            start=(b == 0 and t == 0), stop=(t == pt - 1))
                    if b == 0:
                        if w % 2 == 0:
                            nc.scalar.copy(
                                out=agg0[:, w * 256:(w + 1) * 256],
                                in_=rps[:])
                        else:
                            nc.vector.tensor_copy(
                                out=agg0[:, w * 256:(w + 1) * 256],
                                in_=rps[:])
                    else:
                        rrm = epool.tile([64, 256], dt.float32, tag="rrm")
                        nc.vector.tensor_copy(out=rrm[:], in_=rps[:])
                        for j2 in range(2):
                            tp = npsum.tile([128, 512], dt.float32,
                                            tag="nps")
                            nc.tensor.transpose(
                                out=tp[:, 0:64],
                                in_=rrm[:, j2 * 128:(j2 + 1) * 128],
                                identity=ident[0:64, 0:64])
                            nc.scalar.copy(
                                out=fm(rh_fm, j2, w * WN, WN),
                                in_=tp[:, 0:64])
                        interleave(w)

            def _emit_body():
                # ---- embed ----
                for j2 in range(2):
                    for cc, cw in chunks:
                        ps = npsum.tile([128, 512], dt.float32, tag="nps")
                        nc.tensor.matmul(
                            out=ps[:, :cw],
                            lhsT=embw_sb[:, j2 * 128:(j2 + 1) * 128],
                            rhs=af_sb[:, cc:cc + cw],
                            start=True, stop=False)
                        nc.tensor.matmul(
                            out=ps[:, :cw],
                            lhsT=embb_sb[:, j2 * 128:(j2 + 1) * 128],
                            rhs=mask_sb[:, cc:cc + cw],
                            start=False, stop=True)
                        nc.scalar.copy(out=fm(x_a, j2, cc, cw),
                                       in_=ps[:, :cw])

                # prologue: layer-0 weights + P/Q + both AG chunks
                if nlayers > 0:
                    w_cur = stage_weights(0)
                    for ci, (cc, cw) in enumerate(chunks):
                        pq_chunk(0, cc, cw, x_a, w_cur[0], wr_ab[0])
                        if ci == ag0_chunk:
                            allgather(0)
                vred = npool.tile([1, NCH], dt.float32, tag="vred")

                for layer in range(nlayers):
                    x_cur = x_ab[layer % 2]
                    x_nxt = x_ab[(layer + 1) % 2]
                    wrhs = wr_ab[layer % 2]
                    wrhs_nxt = wr_ab[(layer + 1) % 2]
                    w1ab_sb, wu_sb, bias_sb = w_cur

                    if layer + 1 < nlayers:
                        w_cur = stage_weights(layer + 1)

                    def interleave(w, layer=layer, x_cur=x_cur, x_nxt=x_nxt,
                                   wu_sb=wu_sb, bias_sb=bias_sb,
                                   w_nxt=w_cur, wrhs_nxt=wrhs_nxt):
                        for ci in wend_chunks.get(w, []):
                            cc, cw = chunks[ci]
                            node_chunk(layer, cc, cw, x_cur, x_nxt,
                                       wu_sb, bias_sb)
                            if layer + 1 < nlayers:
                                pq_chunk(layer + 1, cc, cw, x_nxt,
                                         w_nxt[0], wrhs_nxt)
                                if ci == ag0_chunk:
                                    allgather(0)
                            else:
                                readout_chunk(ci, cc, cw, x_nxt, vred)

                    if skip_edge:
                        for w in range(nwin):
                            for j2 in range(2):
                                nc.gpsimd.memset(
                                    fm(rh_fm, j2, w * WN, WN), 0)
                            interleave(w)
                    else:
                        edge_pass(layer, 0, wrhs, lambda w: None)
                        edge_pass(layer, 1, wrhs, interleave)

                if nlayers == 0:
                    for ci, (cc, cw) in enumerate(chunks):
                        readout_chunk(ci, cc, cw, x_a, vred)

                psum_sb = npool.tile([1, 1], dt.float32, tag="psc")
                nc.vector.tensor_reduce(
                    out=psum_sb[:], in_=vred[:],
                    axis=mybir.AxisListType.X, op=ALU.add)
                nc.sync.dma_start(out=partial_out[:], in_=psum_sb[:])

            for _rep in range(reps):
                _emit_body()

    nc.compile()
    return nc


# ----------------------------------------------------------------------------
# Entry point
# ----------------------------------------------------------------------------
def kernel(**inputs) -> np.ndarray:
    from concourse.bass_utils import run_bass_kernel_spmd

    edge_index = np.asarray(inputs["edge_index"])
    plan = _plan(edge_index, np.asarray(inputs["edge_features"], np.float32),
                 np.asarray(inputs["atom_features"], np.float32))
    wts = _pack_weights(inputs)

    nlayers = int(os.environ.get("KERNEL_LAYERS", str(NLAYERS)))
    skip_edge = bool(int(os.environ.get("KERNEL_SKIP_EDGE", "0")))
    skip_gather = bool(int(os.environ.get("KERNEL_SKIP_GATHER", "0")))
    skip_coll = bool(int(os.environ.get("KERNEL_SKIP_COLL", "0")))
    key = (plan["nwin"], plan["W0"], nlayers, skip_edge, skip_gather,
           skip_coll)
    if key not in _cache:
        import time as _t
        t0 = _t.time()
        _cache[key] = _build(plan["nwin"], plan["W0"], nlayers=nlayers,
                             skip_edge=skip_edge, skip_gather=skip_gather,
                             skip_coll=skip_coll)
        print(f"build+schedule: {_t.time() - t0:.1f}s", flush=True)
    nc = _cache[key]

    in_maps = build_inmaps(plan, wts)
    res = run_bass_kernel_spmd(nc, in_maps, list(range(N_CORES)))
    total = sum(float(res.results[c]["partial"][0, 0])
                for c in range(N_CORES))
    out = np.float32(total / N_NODES) + np.asarray(inputs["r_b3"],
                                                   np.float32).reshape(1)
    return out.astype(np.float32)


def build_inmaps(plan, wts):
    shared = dict(w1ab=wts["w1ab"], w1cb=wts["w1cb"], wu=wts["wu"],
                  bias=wts["bias"], embw=wts["embw"], embb=wts["embb"],
                  rw1=wts["rw1"], rb1=wts["rb1"], rw2=wts["rw2"],
                  rb2=wts["rb2"], rw3=wts["rw3"])
    in_maps = []
    for c in range(N_CORES):
        pc = plan["per_core"][c]
        in_maps.append({**shared, "comb": pc["comb"], "scat": pc["scat"],
                        "srcw": pc["srcw"], "af": pc["af"], "deg": pc["deg"],
                        "mask": pc["mask"]})
    return in_maps
